# revision 24
# baseline (speedup 1.0000x reference)
"""Trainium2 Bass kernel for BimanualSpatialAttention.

Data-parallel over batch: 8 cores x 64 batch rows each.

Per-core layout strategy (B=64, T=4096, F=16):
  - x is fed twice in bf16: feature-major XF (for PE conv/matmul rhs) and
    token-major XT (for the output path), both pre-permuted host-side so
    every DMA is large and contiguous.
  - conv + MLP run as block-diagonal batch-packed bf16 matmuls in
    feature-major layout (8 batches packed across 128 partitions).
  - LayerNorm stats are derived from per-arm sums S=sum(x_arm) and
    Q=sum(x_arm^2) (PE ones-matmuls), transposed to token-major together
    with the alphas via PE transpose, then combined on DVE:
       mu  = (a0*S_L + a1*S_R)/16,  a = 1+alpha
       var = (a0^2*Q_L + a1^2*Q_R)/16 - mu^2
       out = x*a*rstd - mu*rstd
"""

import numpy as np
import ml_dtypes

EPS = 1e-5
B_FULL, T, F = 512, 4096, 16
NCORES = 8
B = B_FULL // NCORES  # 64 per core
NG = 8   # batch groups per core (8 batches each)
NW = 8   # token windows per group
W = 512  # tokens per window
NC_CH = 4  # 128-token chunks per window

_CACHE = {}


def _build_nc():
    import concourse.bacc as bacc
    import concourse.bass as bass
    import concourse.tile as tile
    from concourse import mybir

    f32 = mybir.dt.float32
    bf16 = mybir.dt.bfloat16
    AF = mybir.ActivationFunctionType
    ALU = mybir.AluOpType
    AX = mybir.AxisListType

    nc = bacc.Bacc("TRN2", target_bir_lowering=False)

    # ---- I/O ----
    xf_d = nc.declare_dram_parameter("xf", [NG, 128, T], bf16, isOutput=False)
    xt_d = nc.declare_dram_parameter("xt", [NG, NW, 128, NC_CH, 8, F], bf16, isOutput=False)
    conv_lhsT = nc.declare_dram_parameter("conv_lhsT", [128, 3, 128], bf16, isOutput=False)
    w1_lhsT = nc.declare_dram_parameter("w1_lhsT", [128, 64], bf16, isOutput=False)
    w2_lhsT = nc.declare_dram_parameter("w2_lhsT", [128, 16], bf16, isOutput=False)
    ones_lhsT = nc.declare_dram_parameter("ones_lhsT", [128, 16], bf16, isOutput=False)
    convb_d = nc.declare_dram_parameter("convb_pat", [128, 1], f32, isOutput=False)
    b1_d = nc.declare_dram_parameter("b1_pat", [128, 1], f32, isOutput=False)
    b2d_d = nc.declare_dram_parameter("b2d_pat", [16, 1], f32, isOutput=False)
    ident_d = nc.declare_dram_parameter("ident", [128, 128], bf16, isOutput=False)
    out_d = nc.declare_dram_parameter("out", [NG, NW, 128, NC_CH, 8, F], f32, isOutput=True)
    alp_d = nc.declare_dram_parameter("alp", [NG, 16, T], bf16, isOutput=True)

    with tile.TileContext(nc) as tc:
        with (
            tc.tile_pool(name="singles", bufs=1) as singles,
            tc.tile_pool(name="io", bufs=4) as io,
            tc.tile_pool(name="xtp", bufs=10) as xtp,
            tc.tile_pool(name="mid", bufs=4) as mid,
            tc.tile_pool(name="soup", bufs=3) as soup,
            tc.tile_pool(name="ph", bufs=1, space="PSUM") as ph_pool,
            tc.tile_pool(name="phid", bufs=2, space="PSUM") as phid_pool,
            tc.tile_pool(name="pmisc", bufs=2, space="PSUM") as pmisc_pool,
            tc.tile_pool(name="ppackt", bufs=2, space="PSUM") as ppackt_pool,
        ):
            # constants
            sb_conv = singles.tile([128, 3, 128], bf16)
            nc.sync.dma_start(out=sb_conv, in_=conv_lhsT[:])
            sb_w1 = singles.tile([128, 64], bf16)
            nc.sync.dma_start(out=sb_w1, in_=w1_lhsT[:])
            sb_w2 = singles.tile([128, 16], bf16)
            nc.sync.dma_start(out=sb_w2, in_=w2_lhsT[:])
            sb_ones = singles.tile([128, 16], bf16)
            nc.sync.dma_start(out=sb_ones, in_=ones_lhsT[:])
            sb_convb = singles.tile([128, 1], f32)
            nc.sync.dma_start(out=sb_convb, in_=convb_d[:])
            sb_b1 = singles.tile([128, 1], f32)
            nc.sync.dma_start(out=sb_b1, in_=b1_d[:])
            sb_b2d = singles.tile([16, 1], f32)
            nc.sync.dma_start(out=sb_b2d, in_=b2d_d[:])
            sb_id = singles.tile([128, 128], bf16)
            nc.sync.dma_start(out=sb_id, in_=ident_d[:])
            sb_eps = singles.tile([128, 1], f32)
            nc.vector.memset(sb_eps, EPS)

            for g in range(NG):
                for hg in range(2):
                    soupsrc = soup.tile([128, 4, NC_CH, 48], bf16, tag="soupsrc")
                    xts = []
                    for wi in range(4):
                        w = hg * 4 + wi
                        t0 = w * W
                        # ---- loads ----
                        xf = io.tile([128, W + 2], bf16, tag="xf")
                        if w == 0:
                            nc.vector.memset(xf[:, 0:2], 0.0)
                            nc.sync.dma_start(out=xf[:, 2:], in_=xf_d[g, :, 0:W])
                        else:
                            nc.sync.dma_start(out=xf, in_=xf_d[g, :, t0 - 2:t0 + W])
                        xt = xtp.tile([128, NC_CH, 8, F], bf16, tag="xt")
                        nc.gpsimd.dma_start(out=xt, in_=xt_d[g, w])
                        xts.append(xt)

                        # ---- conv (3 taps, 2 subgroups of 4 batches) ----
                        ps_h = ph_pool.tile([128, 2, W], f32, tag="ps_h")
                        for s in range(2):
                            for k in range(3):
                                nc.tensor.matmul(
                                    ps_h[:, s, :],
                                    sb_conv[s * 64:(s + 1) * 64, k, :],
                                    xf[s * 64:(s + 1) * 64, k:k + W],
                                    start=(k == 0), stop=(k == 2),
                                )
                        hcat = mid.tile([128, 2, W], bf16, tag="hcat")
                        for s in range(2):
                            nc.scalar.activation(hcat[:, s, :], ps_h[:, s, :],
                                                 AF.Relu, bias=sb_convb)
                        # ---- MLP ----
                        ps_hid = phid_pool.tile([128, W], f32, tag="ps_hid")
                        for s in range(2):
                            nc.tensor.matmul(ps_hid[s * 64:(s + 1) * 64, :],
                                             sb_w1, hcat[:, s, :],
                                             start=True, stop=True)
                        hid = mid.tile([128, W], bf16, tag="hid")
                        if w % 2 == 0:
                            nc.scalar.activation(hid, ps_hid, AF.Relu, bias=sb_b1)
                        else:
                            nc.vector.tensor_scalar(hid, ps_hid, sb_b1, 0.0,
                                                    op0=ALU.add, op1=ALU.max)

                        ps_misc = pmisc_pool.tile([128, W], f32, tag="ps_misc")
                        nc.tensor.matmul(ps_misc[64:80, :], sb_w2, hid,
                                         start=True, stop=True)
                        # ---- S and Q ----
                        nc.tensor.matmul(ps_misc[0:16, :], sb_ones, xf[:, 2:],
                                         start=True, stop=True)
                        xfsq = mid.tile([128, W], bf16, tag="xfsq")
                        nc.vector.tensor_mul(xfsq, xf[:, 2:], xf[:, 2:])
                        nc.tensor.matmul(ps_misc[32:48, :], sb_ones, xfsq,
                                         start=True, stop=True)
                        # ---- pack + sigmoid ----
                        packst = mid.tile([128, W], bf16, tag="packst")
                        nc.vector.tensor_copy(packst[0:48, :], ps_misc[0:48, :])
                        nc.scalar.activation(packst[64:80, :], ps_misc[64:80, :],
                                             AF.Sigmoid, bias=sb_b2d)
                        nc.scalar.dma_start(out=alp_d[g, :, t0:t0 + W],
                                          in_=packst[64:80, :])
                        # ---- transpose to token-major ----
                        packt = ppackt_pool.tile([128, NC_CH, 128], bf16, tag="packt")
                        for c in range(NC_CH):
                            nc.tensor.matmul(packt[:, c, :],
                                             packst[:, c * 128:(c + 1) * 128],
                                             sb_id, is_transpose=True,
                                             start=True, stop=True)
                        nc.vector.tensor_copy(
                            soupsrc[:, wi].rearrange("p c (g x) -> p c g x", x=16),
                            packt[:].rearrange("p c (g x) -> p c g x", x=32)[:, :, 0:3, :16])

                    # ---- LN soup over 8 windows (token-major) ----
                    # packt cols: [0:16]=S(b,arm) [16:32]=junk [32:48]=Q [48:64]=alpha
                    aT = soupsrc[:, :, :, 32:48]
                    ST = soupsrc[:, :, :, 0:16]
                    QT = soupsrc[:, :, :, 16:32]
                    a1 = soup.tile([128, 4, NC_CH, 16], f32, tag="a1")
                    nc.vector.tensor_scalar_add(a1, aT, 1.0)
                    s1 = soup.tile([128, 4, NC_CH, 16], f32, tag="s1")
                    nc.vector.tensor_tensor(s1, a1, ST, op=ALU.mult)
                    mu16 = soup.tile([128, 4, NC_CH, 8], f32, tag="mu16")
                    nc.vector.reduce_sum(
                        mu16, s1[:].rearrange("p w c (b a) -> p w c b a", a=2),
                        axis=AX.X)
                    a2 = soup.tile([128, 4, NC_CH, 16], f32, tag="a2")
                    nc.vector.tensor_tensor(a2, a1, a1, op=ALU.mult)
                    q1 = soup.tile([128, 4, NC_CH, 16], f32, tag="q1")
                    nc.vector.tensor_tensor(q1, a2, QT, op=ALU.mult)
                    q16 = soup.tile([128, 4, NC_CH, 8], f32, tag="q16")
                    nc.vector.reduce_sum(
                        q16, q1[:].rearrange("p w c (b a) -> p w c b a", a=2),
                        axis=AX.X)
                    mu_ = soup.tile([128, 4, NC_CH, 8], f32, tag="mu_")
                    nc.vector.tensor_scalar_mul(mu_, mu16, 1.0 / 16.0)
                    v = soup.tile([128, 4, NC_CH, 8], f32, tag="v")
                    # v = q16/16 - mu_^2
                    nc.vector.scalar_tensor_tensor(
                        v, mu_, 1.0, mu_, op0=ALU.mult, op1=ALU.mult)
                    nc.vector.scalar_tensor_tensor(
                        v, q16, 1.0 / 16.0, v, op0=ALU.mult, op1=ALU.subtract)
                    sd = soup.tile([128, 4, NC_CH, 8], f32, tag="sd")
                    nc.scalar.activation(sd, v, AF.Sqrt, bias=sb_eps)
                    rstd = soup.tile([128, 4, NC_CH, 8], f32, tag="rstd")
                    nc.vector.reciprocal(rstd, sd)
                    P = soup.tile([128, 4, NC_CH, 16], bf16, tag="P")
                    nc.vector.tensor_tensor(
                        P[:].rearrange("p w c (b a) -> p w c b a", a=2),
                        a1[:].rearrange("p w c (b a) -> p w c b a", a=2),
                        rstd[:].unsqueeze(4).to_broadcast((128, 4, NC_CH, 8, 2)),
                        op=ALU.mult)
                    Cn = soup.tile([128, 4, NC_CH, 8], f32, tag="Cn")
                    nc.vector.scalar_tensor_tensor(
                        Cn, mu_, 1.0, rstd, op0=ALU.mult, op1=ALU.mult)

                    # ---- output path per window ----
                    for wi in range(4):
                        w = hg * 4 + wi
                        xt = xts[wi]
                        o1 = mid.tile([128, NC_CH, 8, F], f32, tag="o1")
                        nc.gpsimd.tensor_mul(
                            o1[:].rearrange("p c b (a f) -> p c b a f", a=2),
                            xt[:].rearrange("p c b (a f) -> p c b a f", a=2),
                            P[:, wi].rearrange("p c (b a) -> p c b a", a=2)
                                .unsqueeze(4).to_broadcast((128, NC_CH, 8, 2, 8)))
                        outt = io.tile([128, NC_CH, 8, F], f32, tag="outt")
                        nc.vector.tensor_tensor(
                            outt, o1,
                            Cn[:, wi].unsqueeze(3).to_broadcast((128, NC_CH, 8, F)),
                            op=ALU.subtract)
                        nc.sync.dma_start(out=out_d[g, w], in_=outt)
    nc.compile()
    return nc


def _host_consts(conv_w, conv_b, w1, b1, w2, b2):
    bf = ml_dtypes.bfloat16
    conv_blk = np.zeros((64, 3, 128), np.float32)
    for k in range(3):
        for b4 in range(4):
            for arm in range(2):
                # rows: b4*16 + arm*8 + i ; cols: b4*32 + arm*16 + o
                conv_blk[b4 * 16 + arm * 8:b4 * 16 + arm * 8 + 8, k,
                         b4 * 32 + arm * 16:b4 * 32 + arm * 16 + 16] = conv_w[:, :, k].T
    conv_lhsT = np.concatenate([conv_blk, conv_blk], axis=0)  # (128, 3, 128)
    w1_lhsT = np.zeros((128, 64), np.float32)
    for b4 in range(4):
        w1_lhsT[b4 * 32:(b4 + 1) * 32, b4 * 16:(b4 + 1) * 16] = w1
    w2d = (w2[:, 0] - w2[:, 1]).astype(np.float32)
    w2_lhsT = np.zeros((128, 16), np.float32)
    for b in range(8):
        w2_lhsT[b * 16:(b + 1) * 16, b * 2] = w2d
        w2_lhsT[b * 16:(b + 1) * 16, b * 2 + 1] = -w2d
    ones_lhsT = np.zeros((128, 16), np.float32)
    for b in range(8):
        for arm in range(2):
            ones_lhsT[b * 16 + arm * 8:b * 16 + arm * 8 + 8, b * 2 + arm] = 1.0
    convb_pat = np.tile(conv_b, 8).reshape(128, 1).astype(np.float32)
    b1_pat = np.tile(b1, 8).reshape(128, 1).astype(np.float32)
    b2dv = float(b2[0] - b2[1])
    b2d_pat = np.array([b2dv, -b2dv] * 8, np.float32).reshape(16, 1)
    ident = np.eye(128, dtype=np.float32)
    return {
        "conv_lhsT": conv_lhsT.astype(bf),
        "w1_lhsT": w1_lhsT.astype(bf),
        "w2_lhsT": w2_lhsT.astype(bf),
        "ones_lhsT": ones_lhsT.astype(bf),
        "convb_pat": convb_pat,
        "b1_pat": b1_pat,
        "b2d_pat": b2d_pat,
        "ident": ident.astype(bf),
    }


def kernel(x, conv_w, conv_b, w1, b1, w2, b2, ln_g, ln_b, _return_results=False,
           _trace=False):
    from concourse.bass_utils import run_bass_kernel_spmd

    bf = ml_dtypes.bfloat16
    x = np.asarray(x, np.float32)
    assert np.allclose(np.asarray(ln_g), 1.0) and np.allclose(np.asarray(ln_b), 0.0), \
        "kernel specialized to ln_g=1, ln_b=0"

    consts = _host_consts(np.asarray(conv_w, np.float32), np.asarray(conv_b, np.float32),
                          np.asarray(w1, np.float32), np.asarray(b1, np.float32),
                          np.asarray(w2, np.float32), np.asarray(b2, np.float32))

    if "nc" not in _CACHE:
        _CACHE["nc"] = _build_nc()
    nc = _CACHE["nc"]

    xbf = x.astype(bf)
    in_maps = []
    for c in range(NCORES):
        xc = xbf[c * B:(c + 1) * B]  # (64, T, 16)
        xg = xc.reshape(NG, 8, T, F)
        # feature-major: xf[g, b*16+f, t]
        xf = np.ascontiguousarray(xg.transpose(0, 1, 3, 2)).reshape(NG, 128, T)
        # token-major: xt[g, w, p, c, b, f]
        xt = np.ascontiguousarray(
            xc.reshape(NG, 8, NW, NC_CH, 128, F).transpose(0, 2, 4, 3, 1, 5))
        m = {"xf": xf, "xt": xt}
        m.update(consts)
        in_maps.append(m)

    res = run_bass_kernel_spmd(nc, in_maps, core_ids=list(range(NCORES)),
                               trace=_trace)
    outs = []
    alps = []
    for c in range(NCORES):
        r = res.results[c]
        o = r["out"]  # (NG, NW, 128, NC_CH, 8, F) f32
        out_c = np.ascontiguousarray(
            o.transpose(0, 4, 1, 3, 2, 5)).reshape(B, T, F)
        a = np.asarray(r["alp"], np.float32)  # (NG, 16, T)
        alp_c = np.ascontiguousarray(
            a.reshape(NG, 8, 2, T).transpose(0, 1, 3, 2)).reshape(B, T, 2)
        outs.append(out_c)
        alps.append(alp_c)
    out_full = np.concatenate(outs, 0)
    alp_full = np.concatenate(alps, 0)
    if _return_results:
        return (out_full, alp_full), res
    return (out_full, alp_full)


# revision 25
# speedup vs baseline: 1.0201x; 1.0201x over previous
"""Trainium2 Bass kernel for BimanualSpatialAttention.

Data-parallel over batch: 8 cores x 64 batch rows each.

Per-core layout strategy (B=64, T=4096, F=16):
  - x is fed twice in bf16: feature-major XF (for PE conv/matmul rhs) and
    token-major XT (for the output path), both pre-permuted host-side so
    every DMA is large and contiguous.
  - conv + MLP run as block-diagonal batch-packed bf16 matmuls in
    feature-major layout (8 batches packed across 128 partitions).
  - LayerNorm stats are derived from per-arm sums S=sum(x_arm) and
    Q=sum(x_arm^2) (PE ones-matmuls), transposed to token-major together
    with the alphas via PE transpose, then combined on DVE:
       mu  = (a0*S_L + a1*S_R)/16,  a = 1+alpha
       var = (a0^2*Q_L + a1^2*Q_R)/16 - mu^2
       out = x*a*rstd - mu*rstd
"""

import numpy as np
import ml_dtypes

EPS = 1e-5
B_FULL, T, F = 512, 4096, 16
NCORES = 8
B = B_FULL // NCORES  # 64 per core
NG = 8   # batch groups per core (8 batches each)
NW = 8   # token windows per group
W = 512  # tokens per window
NC_CH = 4  # 128-token chunks per window

_CACHE = {}


def _build_nc():
    import concourse.bacc as bacc
    import concourse.bass as bass
    import concourse.tile as tile
    from concourse import mybir

    f32 = mybir.dt.float32
    bf16 = mybir.dt.bfloat16
    AF = mybir.ActivationFunctionType
    ALU = mybir.AluOpType
    AX = mybir.AxisListType

    nc = bacc.Bacc("TRN2", target_bir_lowering=False)

    # ---- I/O ----
    xf_d = nc.declare_dram_parameter("xf", [NG, 128, T], bf16, isOutput=False)
    xt_d = nc.declare_dram_parameter("xt", [NG, NW, 128, NC_CH, 8, F], bf16, isOutput=False)
    conv_lhsT = nc.declare_dram_parameter("conv_lhsT", [128, 3, 128], bf16, isOutput=False)
    w1_lhsT = nc.declare_dram_parameter("w1_lhsT", [128, 64], bf16, isOutput=False)
    w2_lhsT = nc.declare_dram_parameter("w2_lhsT", [128, 16], bf16, isOutput=False)
    ones_lhsT = nc.declare_dram_parameter("ones_lhsT", [128, 16], bf16, isOutput=False)
    convb_d = nc.declare_dram_parameter("convb_pat", [128, 1], f32, isOutput=False)
    b1_d = nc.declare_dram_parameter("b1_pat", [128, 1], f32, isOutput=False)
    b2d_d = nc.declare_dram_parameter("b2d_pat", [16, 1], f32, isOutput=False)
    ident_d = nc.declare_dram_parameter("ident", [128, 128], bf16, isOutput=False)
    out_d = nc.declare_dram_parameter("out", [NG, NW, 128, NC_CH, 8, F], f32, isOutput=True)
    alp_d = nc.declare_dram_parameter("alp", [NG, 16, T], bf16, isOutput=True)

    with tile.TileContext(nc) as tc:
        with (
            tc.tile_pool(name="singles", bufs=1) as singles,
            tc.tile_pool(name="io", bufs=6) as io,
            tc.tile_pool(name="xtp", bufs=10) as xtp,
            tc.tile_pool(name="mid", bufs=6) as mid,
            tc.tile_pool(name="soup", bufs=4) as soup,
            tc.tile_pool(name="ph", bufs=1, space="PSUM") as ph_pool,
            tc.tile_pool(name="phid", bufs=2, space="PSUM") as phid_pool,
            tc.tile_pool(name="pmisc", bufs=2, space="PSUM") as pmisc_pool,
            tc.tile_pool(name="ppackt", bufs=2, space="PSUM") as ppackt_pool,
        ):
            # constants
            sb_conv = singles.tile([128, 3, 128], bf16)
            nc.sync.dma_start(out=sb_conv, in_=conv_lhsT[:])
            sb_w1 = singles.tile([128, 64], bf16)
            nc.sync.dma_start(out=sb_w1, in_=w1_lhsT[:])
            sb_w2 = singles.tile([128, 16], bf16)
            nc.sync.dma_start(out=sb_w2, in_=w2_lhsT[:])
            sb_ones = singles.tile([128, 16], bf16)
            nc.sync.dma_start(out=sb_ones, in_=ones_lhsT[:])
            sb_convb = singles.tile([128, 1], f32)
            nc.sync.dma_start(out=sb_convb, in_=convb_d[:])
            sb_b1 = singles.tile([128, 1], f32)
            nc.sync.dma_start(out=sb_b1, in_=b1_d[:])
            sb_b2d = singles.tile([16, 1], f32)
            nc.sync.dma_start(out=sb_b2d, in_=b2d_d[:])
            sb_id = singles.tile([128, 128], bf16)
            nc.sync.dma_start(out=sb_id, in_=ident_d[:])
            sb_eps = singles.tile([128, 1], f32)
            nc.vector.memset(sb_eps, EPS)

            for g in range(NG):
                for hg in range(2):
                    soupsrc = soup.tile([128, 4, NC_CH, 48], bf16, tag="soupsrc")
                    xts = []
                    for wi in range(4):
                        w = hg * 4 + wi
                        t0 = w * W
                        # ---- loads ----
                        xf = io.tile([128, W + 2], bf16, tag="xf")
                        if w == 0:
                            nc.vector.memset(xf[:, 0:2], 0.0)
                            nc.sync.dma_start(out=xf[:, 2:], in_=xf_d[g, :, 0:W])
                        else:
                            nc.sync.dma_start(out=xf, in_=xf_d[g, :, t0 - 2:t0 + W])
                        xt = xtp.tile([128, NC_CH, 8, F], bf16, tag="xt")
                        nc.gpsimd.dma_start(out=xt, in_=xt_d[g, w])
                        xts.append(xt)

                        # ---- conv (3 taps, 2 subgroups of 4 batches) ----
                        ps_h = ph_pool.tile([128, 2, W], f32, tag="ps_h")
                        for s in range(2):
                            for k in range(3):
                                nc.tensor.matmul(
                                    ps_h[:, s, :],
                                    sb_conv[s * 64:(s + 1) * 64, k, :],
                                    xf[s * 64:(s + 1) * 64, k:k + W],
                                    start=(k == 0), stop=(k == 2),
                                )
                        hcat = mid.tile([128, 2, W], bf16, tag="hcat")
                        for s in range(2):
                            nc.scalar.activation(hcat[:, s, :], ps_h[:, s, :],
                                                 AF.Relu, bias=sb_convb)
                        # ---- MLP ----
                        ps_hid = phid_pool.tile([128, W], f32, tag="ps_hid")
                        for s in range(2):
                            nc.tensor.matmul(ps_hid[s * 64:(s + 1) * 64, :],
                                             sb_w1, hcat[:, s, :],
                                             start=True, stop=True)
                        hid = mid.tile([128, W], bf16, tag="hid")
                        if w % 2 == 0:
                            nc.scalar.activation(hid, ps_hid, AF.Relu, bias=sb_b1)
                        else:
                            nc.vector.tensor_scalar(hid, ps_hid, sb_b1, 0.0,
                                                    op0=ALU.add, op1=ALU.max)

                        ps_misc = pmisc_pool.tile([128, W], f32, tag="ps_misc")
                        nc.tensor.matmul(ps_misc[64:80, :], sb_w2, hid,
                                         start=True, stop=True)
                        # ---- S and Q ----
                        nc.tensor.matmul(ps_misc[0:16, :], sb_ones, xf[:, 2:],
                                         start=True, stop=True)
                        xfsq = mid.tile([128, W], bf16, tag="xfsq")
                        nc.vector.tensor_mul(xfsq, xf[:, 2:], xf[:, 2:])
                        nc.tensor.matmul(ps_misc[32:48, :], sb_ones, xfsq,
                                         start=True, stop=True)
                        # ---- pack + sigmoid ----
                        packst = mid.tile([128, W], bf16, tag="packst")
                        nc.vector.tensor_copy(packst[0:48, :], ps_misc[0:48, :])
                        nc.scalar.activation(packst[64:80, :], ps_misc[64:80, :],
                                             AF.Sigmoid, bias=sb_b2d)
                        nc.sync.dma_start(out=alp_d[g, :, t0:t0 + W],
                                          in_=packst[64:80, :])
                        # ---- transpose to token-major ----
                        packt = ppackt_pool.tile([128, NC_CH, 128], bf16, tag="packt")
                        for c in range(NC_CH):
                            nc.tensor.matmul(packt[:, c, :],
                                             packst[:, c * 128:(c + 1) * 128],
                                             sb_id, is_transpose=True,
                                             start=True, stop=True)
                        nc.vector.tensor_copy(
                            soupsrc[:, wi].rearrange("p c (g x) -> p c g x", x=16),
                            packt[:].rearrange("p c (g x) -> p c g x", x=32)[:, :, 0:3, :16])

                    # ---- LN soup over 8 windows (token-major) ----
                    # packt cols: [0:16]=S(b,arm) [16:32]=junk [32:48]=Q [48:64]=alpha
                    aT = soupsrc[:, :, :, 32:48]
                    ST = soupsrc[:, :, :, 0:16]
                    QT = soupsrc[:, :, :, 16:32]
                    a1 = soup.tile([128, 4, NC_CH, 16], f32, tag="a1")
                    nc.vector.tensor_scalar_add(a1, aT, 1.0)
                    s1 = soup.tile([128, 4, NC_CH, 16], f32, tag="s1")
                    nc.vector.tensor_tensor(s1, a1, ST, op=ALU.mult)
                    mu16 = soup.tile([128, 4, NC_CH, 8], f32, tag="mu16")
                    nc.vector.reduce_sum(
                        mu16, s1[:].rearrange("p w c (b a) -> p w c b a", a=2),
                        axis=AX.X)
                    a2 = soup.tile([128, 4, NC_CH, 16], f32, tag="a2")
                    nc.vector.tensor_tensor(a2, a1, a1, op=ALU.mult)
                    q1 = soup.tile([128, 4, NC_CH, 16], f32, tag="q1")
                    nc.vector.tensor_tensor(q1, a2, QT, op=ALU.mult)
                    q16 = soup.tile([128, 4, NC_CH, 8], f32, tag="q16")
                    nc.vector.reduce_sum(
                        q16, q1[:].rearrange("p w c (b a) -> p w c b a", a=2),
                        axis=AX.X)
                    mu_ = soup.tile([128, 4, NC_CH, 8], f32, tag="mu_")
                    nc.vector.tensor_scalar_mul(mu_, mu16, 1.0 / 16.0)
                    v = soup.tile([128, 4, NC_CH, 8], f32, tag="v")
                    # v = q16/16 - mu_^2
                    nc.vector.scalar_tensor_tensor(
                        v, mu_, 1.0, mu_, op0=ALU.mult, op1=ALU.mult)
                    nc.vector.scalar_tensor_tensor(
                        v, q16, 1.0 / 16.0, v, op0=ALU.mult, op1=ALU.subtract)
                    sd = soup.tile([128, 4, NC_CH, 8], f32, tag="sd")
                    nc.scalar.activation(sd, v, AF.Sqrt, bias=sb_eps)
                    rstd = soup.tile([128, 4, NC_CH, 8], f32, tag="rstd")
                    nc.vector.reciprocal(rstd, sd)
                    P = soup.tile([128, 4, NC_CH, 16], bf16, tag="P")
                    nc.vector.tensor_tensor(
                        P[:].rearrange("p w c (b a) -> p w c b a", a=2),
                        a1[:].rearrange("p w c (b a) -> p w c b a", a=2),
                        rstd[:].unsqueeze(4).to_broadcast((128, 4, NC_CH, 8, 2)),
                        op=ALU.mult)
                    Cn = soup.tile([128, 4, NC_CH, 8], f32, tag="Cn")
                    nc.vector.scalar_tensor_tensor(
                        Cn, mu_, 1.0, rstd, op0=ALU.mult, op1=ALU.mult)

                    # ---- output path per window ----
                    for wi in range(4):
                        w = hg * 4 + wi
                        xt = xts[wi]
                        o1 = mid.tile([128, NC_CH, 8, F], f32, tag="o1")
                        nc.gpsimd.tensor_mul(
                            o1[:].rearrange("p c b (a f) -> p c b a f", a=2),
                            xt[:].rearrange("p c b (a f) -> p c b a f", a=2),
                            P[:, wi].rearrange("p c (b a) -> p c b a", a=2)
                                .unsqueeze(4).to_broadcast((128, NC_CH, 8, 2, 8)))
                        outt = io.tile([128, NC_CH, 8, F], f32, tag="outt")
                        nc.vector.tensor_tensor(
                            outt, o1,
                            Cn[:, wi].unsqueeze(3).to_broadcast((128, NC_CH, 8, F)),
                            op=ALU.subtract)
                        nc.sync.dma_start(out=out_d[g, w], in_=outt)
    nc.compile()
    return nc


def _host_consts(conv_w, conv_b, w1, b1, w2, b2):
    bf = ml_dtypes.bfloat16
    conv_blk = np.zeros((64, 3, 128), np.float32)
    for k in range(3):
        for b4 in range(4):
            for arm in range(2):
                # rows: b4*16 + arm*8 + i ; cols: b4*32 + arm*16 + o
                conv_blk[b4 * 16 + arm * 8:b4 * 16 + arm * 8 + 8, k,
                         b4 * 32 + arm * 16:b4 * 32 + arm * 16 + 16] = conv_w[:, :, k].T
    conv_lhsT = np.concatenate([conv_blk, conv_blk], axis=0)  # (128, 3, 128)
    w1_lhsT = np.zeros((128, 64), np.float32)
    for b4 in range(4):
        w1_lhsT[b4 * 32:(b4 + 1) * 32, b4 * 16:(b4 + 1) * 16] = w1
    w2d = (w2[:, 0] - w2[:, 1]).astype(np.float32)
    w2_lhsT = np.zeros((128, 16), np.float32)
    for b in range(8):
        w2_lhsT[b * 16:(b + 1) * 16, b * 2] = w2d
        w2_lhsT[b * 16:(b + 1) * 16, b * 2 + 1] = -w2d
    ones_lhsT = np.zeros((128, 16), np.float32)
    for b in range(8):
        for arm in range(2):
            ones_lhsT[b * 16 + arm * 8:b * 16 + arm * 8 + 8, b * 2 + arm] = 1.0
    convb_pat = np.tile(conv_b, 8).reshape(128, 1).astype(np.float32)
    b1_pat = np.tile(b1, 8).reshape(128, 1).astype(np.float32)
    b2dv = float(b2[0] - b2[1])
    b2d_pat = np.array([b2dv, -b2dv] * 8, np.float32).reshape(16, 1)
    ident = np.eye(128, dtype=np.float32)
    return {
        "conv_lhsT": conv_lhsT.astype(bf),
        "w1_lhsT": w1_lhsT.astype(bf),
        "w2_lhsT": w2_lhsT.astype(bf),
        "ones_lhsT": ones_lhsT.astype(bf),
        "convb_pat": convb_pat,
        "b1_pat": b1_pat,
        "b2d_pat": b2d_pat,
        "ident": ident.astype(bf),
    }


def kernel(x, conv_w, conv_b, w1, b1, w2, b2, ln_g, ln_b, _return_results=False,
           _trace=False):
    from concourse.bass_utils import run_bass_kernel_spmd

    bf = ml_dtypes.bfloat16
    x = np.asarray(x, np.float32)
    assert np.allclose(np.asarray(ln_g), 1.0) and np.allclose(np.asarray(ln_b), 0.0), \
        "kernel specialized to ln_g=1, ln_b=0"

    consts = _host_consts(np.asarray(conv_w, np.float32), np.asarray(conv_b, np.float32),
                          np.asarray(w1, np.float32), np.asarray(b1, np.float32),
                          np.asarray(w2, np.float32), np.asarray(b2, np.float32))

    if "nc" not in _CACHE:
        _CACHE["nc"] = _build_nc()
    nc = _CACHE["nc"]

    xbf = x.astype(bf)
    in_maps = []
    for c in range(NCORES):
        xc = xbf[c * B:(c + 1) * B]  # (64, T, 16)
        xg = xc.reshape(NG, 8, T, F)
        # feature-major: xf[g, b*16+f, t]
        xf = np.ascontiguousarray(xg.transpose(0, 1, 3, 2)).reshape(NG, 128, T)
        # token-major: xt[g, w, p, c, b, f]
        xt = np.ascontiguousarray(
            xc.reshape(NG, 8, NW, NC_CH, 128, F).transpose(0, 2, 4, 3, 1, 5))
        m = {"xf": xf, "xt": xt}
        m.update(consts)
        in_maps.append(m)

    res = run_bass_kernel_spmd(nc, in_maps, core_ids=list(range(NCORES)),
                               trace=_trace)
    outs = []
    alps = []
    for c in range(NCORES):
        r = res.results[c]
        o = r["out"]  # (NG, NW, 128, NC_CH, 8, F) f32
        out_c = np.ascontiguousarray(
            o.transpose(0, 4, 1, 3, 2, 5)).reshape(B, T, F)
        a = np.asarray(r["alp"], np.float32)  # (NG, 16, T)
        alp_c = np.ascontiguousarray(
            a.reshape(NG, 8, 2, T).transpose(0, 1, 3, 2)).reshape(B, T, 2)
        outs.append(out_c)
        alps.append(alp_c)
    out_full = np.concatenate(outs, 0)
    alp_full = np.concatenate(alps, 0)
    if _return_results:
        return (out_full, alp_full), res
    return (out_full, alp_full)
